# revision 4
# baseline (speedup 1.0000x reference)
"""Trainium2 Bass kernel for nn_ATTPool (attention-weighted temporal pooling).

Reference math (per batch b):
    att = (x_tre[b] + pos) @ W.T + bias              # (T=32, C=64)
    a   = softmax_T(att)                             # softmax over T
    out = sum_t a[t,c] * x[b,c,t,:,:] + x[b,c,T-1,:,:]   # (C, H*W)

pos-enc rows and the bias are constant along T (the softmax axis), so they
cancel exactly inside softmax: a = softmax_T(x_tre[b] @ W.T).  The
+x[:,:,-1] residual folds into the pooling weights as +1 at t=T-1.

Strategy: data-parallel over B=8 across the 8 NeuronCores.  Per core the
dominant work is streaming x[b] (32 MiB) once through TensorE: x[b] viewed
as (C*T=2048, HW=4096) is split into 16 chunks of 128 rows (4 channels x
32 timesteps).  A sparse (128, 64) weight tile routes row (c,t) to output
partition c with weight a+[t,c]; one matmul group per chunk accumulates
into a persistent (64, 4096) PSUM accumulator (start on k=0, stop on
k=15).  Epilogue: per-bank PSUM -> SBUF -> HBM.

Hardware notes that shaped the code:
  * A Matmult can carry at most ONE semaphore wait (walrus setupSyncWait
    limit), so dependencies are funneled: both attention operands arrive
    via a single DMA; a dummy 1x1 matmul absorbs the DVE wait before the
    main loop; each chunk's matmuls then wait only on their x-chunk DMA.
  * DVE cannot move data across partitions, so the 4x partition-group
    replication of the attention row block goes through a tiny DRAM
    round-trip (SBUF -> DRAM -> 4 strided loads).
  * DMA cannot touch PSUM, hence the ScalarE copies in the epilogue.
"""

import threading
from contextlib import ExitStack

import numpy as np

import concourse.bacc as bacc
import concourse.bass as bass
import concourse.tile as tile
from concourse import mybir
from concourse.bass_utils import run_bass_kernel_spmd

F32 = mybir.dt.float32

B, C, T = 8, 64, 32
HW = 64 * 64                 # 4096
CT = C * T                   # 2048
NCHUNK = 16                  # chunks of 128 (c,t)-rows
NBANK = 8                    # 512-f32 matmul slices of the 4096 free dim
BANK = HW // NBANK           # 512
X_BUFS = 4                   # in-flight x chunk tiles (2 MiB each)


def _build_nc() -> bass.Bass:
    nc = bacc.Bacc(None, target_bir_lowering=False)

    xb = nc.dram_tensor("xb", [CT, HW], F32, kind="ExternalInput")
    # pre[:, 0:64] = W.T, pre[:, 64:96] = x_tre[b].T  (transposed on host)
    pre = nc.dram_tensor("pre", [C, C + T], F32, kind="ExternalInput")
    out = nc.dram_tensor("out", [C, HW], F32, kind="ExternalOutput")

    with ExitStack() as ctx:
        tc = ctx.enter_context(tile.TileContext(nc))
        consts = ctx.enter_context(tc.tile_pool(name="consts", bufs=1))
        xpool = ctx.enter_context(tc.tile_pool(name="xp", bufs=X_BUFS))
        psum = ctx.enter_context(
            tc.tile_pool(name="ps", bufs=1, space=bass.MemorySpace.PSUM)
        )
        dram = ctx.enter_context(
            tc.tile_pool(name="dr", bufs=1, space=bass.MemorySpace.DRAM)
        )

        # [0:64, :] is the pooled-output accumulator; [0:64, 0:32] doubles
        # as scratch for the attention matmul (consumed before chunk 0).
        acc = psum.tile([128, HW], F32)

        pre_sb = consts.tile([C, C + T], F32)
        nc.scalar.dma_start(out=pre_sb[:], in_=pre[:])

        # --- attention preamble (tiny; overlaps chunk-0 x DMA) -----------
        # att^T[c, t] = sum_i W[c,i] * x_tre[t,i]
        att_ps = acc[0:C, 0:T]
        nc.tensor.matmul(att_ps, pre_sb[:, 0:C], pre_sb[:, C : C + T])
        att_sb = consts.tile([C, T], F32)
        nc.vector.tensor_copy(out=att_sb[:], in_=att_ps)

        # softmax over t (free dim): exp(x - max)/sum
        negm = consts.tile([C, 1], F32)
        nc.vector.tensor_reduce(
            negm[:], att_sb[:], axis=mybir.AxisListType.X,
            op=mybir.AluOpType.max, negate=True,
        )
        exp_ct = consts.tile([C, T], F32)
        ssum = consts.tile([C, 1], F32)
        nc.scalar.activation(
            out=exp_ct[:], in_=att_sb[:],
            func=mybir.ActivationFunctionType.Exp,
            bias=negm[:], scale=1.0, accum_out=ssum[:],
        )
        rsum = consts.tile([C, 1], F32)
        nc.vector.reciprocal(rsum[:], ssum[:])
        attn = consts.tile([C, T], F32)
        nc.vector.tensor_scalar_mul(attn[:], exp_ct[:], rsum[:])
        # residual +x[:,:,T-1]  ==  weight +1 at t = T-1
        nc.vector.tensor_scalar_add(attn[:, T - 1 : T], attn[:, T - 1 : T], 1.0)

        # 32x32-block transpose: vt[32h+t, c'] = attn[32h+c', t]
        vt = consts.tile([C, T], F32)
        nc.vector.transpose(vt[:], attn[:])

        # Cross-partition replicate via DRAM round-trip:
        # rep[32r+t, c] = a+[t, c] = vt_dram[32*(c//32) + t, c%32]
        vt_dram = dram.tile([C, T], F32)
        nc.scalar.dma_start(out=vt_dram[:], in_=vt[:])
        rep_sb = consts.tile([128, C], F32)
        for r in range(4):
            nc.scalar.dma_start(
                out=rep_sb[32 * r : 32 * (r + 1), :].rearrange(
                    "t (h c) -> t h c", h=2
                ),
                in_=vt_dram[:].rearrange("(h t) c -> t h c", h=2),
            )

        # Sparse routing weights for all 16 chunks: chunk k is
        # lhsT_big[:, 64k:64k+64]; its column c = 4k+j is nonzero on
        # partitions [32j, 32j+32) with values a+[t, 4k+j].  Writing
        # rep[32j+t, j+4m] -> lhsT_big[32j+t, j+68m] places exactly those.
        lhsT_big = consts.tile([128, NCHUNK * C], F32)
        nc.vector.memset(lhsT_big[:], 0.0)
        for j in range(4):
            nc.vector.tensor_copy(
                out=lhsT_big[32 * j : 32 * (j + 1), j :: 68],
                in_=rep_sb[32 * j : 32 * (j + 1), j : C : 4],
            )

        # Dummy matmul: absorbs the DVE wait (lhsT_big ready + acc-scratch
        # WAR) so each main matmul needs at most one wait (its x DMA).
        # Columns 0:4 are written by all four strided copies (m=0 strips).
        nc.tensor.matmul(acc[0:4, 0:1], lhsT_big[:, 0:4], lhsT_big[:, 0:1])

        # --- main loop: stream x[b] through TensorE ----------------------
        for k in range(NCHUNK):
            xtile = xpool.tile([128, HW], F32)
            nc.sync.dma_start(out=xtile[:], in_=xb[128 * k : 128 * (k + 1), :])
            for n in range(NBANK):
                nc.tensor.matmul(
                    acc[0:C, BANK * n : BANK * (n + 1)],
                    lhsT_big[:, C * k : C * (k + 1)],
                    xtile[:, BANK * n : BANK * (n + 1)],
                    start=(k == 0),
                    stop=(k == NCHUNK - 1),
                )

        # --- epilogue: per-bank PSUM -> SBUF -> HBM ----------------------
        out_sb = consts.tile([C, HW], F32)
        for n in range(NBANK):
            nc.scalar.copy(
                out=out_sb[:, BANK * n : BANK * (n + 1)],
                in_=acc[0:C, BANK * n : BANK * (n + 1)],
            )
            nc.sync.dma_start(
                out=out[:, BANK * n : BANK * (n + 1)],
                in_=out_sb[:, BANK * n : BANK * (n + 1)],
            )

    nc.compile()
    return nc


_NC_LOCK = threading.Lock()
_NC_CACHE: list = []


def _get_nc() -> bass.Bass:
    with _NC_LOCK:
        if not _NC_CACHE:
            _NC_CACHE.append(_build_nc())
        return _NC_CACHE[0]


def run(x, x_tre, W, b=None, trace: bool = False):
    """Run the SPMD kernel on 8 cores; returns (BassKernelResults, output)."""
    x = np.asarray(x, dtype=np.float32)
    x_tre = np.asarray(x_tre, dtype=np.float32)
    WT = np.ascontiguousarray(np.asarray(W, dtype=np.float32).T)
    maps = []
    for core in range(B):
        pre = np.concatenate([WT, np.asarray(x_tre[core], np.float32).T], axis=1)
        maps.append(
            {
                "xb": np.ascontiguousarray(x[core]).reshape(CT, HW),
                "pre": np.ascontiguousarray(pre),
            }
        )
    nc = _get_nc()
    res = run_bass_kernel_spmd(nc, maps, core_ids=list(range(B)), trace=trace)
    outs = np.stack([np.asarray(r["out"]).reshape(C, 64, 64) for r in res.results])
    return res, outs.astype(np.float32)


def kernel(x, x_tre, W, b=None, **_unused):
    _, out = run(x, x_tre, W, b)
    return out
